# revision 1
# baseline (speedup 1.0000x reference)
"""APPNP GNN (MLP encoder + K-hop personalized-pagerank propagation + log_softmax)
distributed across 8 Trainium2 NeuronCores.

Strategy
--------
Nodes are dealt round-robin by descending degree to the 8 cores (load balance +
uniform per-block gather depth). Each core owns a node shard; the propagation
state u = dinv * out (symmetric-norm folded into per-node scale factors) is
only [N, 64] so every hop we AllGather the bf16 state into a replicated DRAM
table and each core does batched indirect-DMA gathers (one per 128-node block,
slot-padded to the block max degree with pointers at a zero row), a strided
DVE reduction over slots, and a tiny scale+add update:
    u' = c1 * segment_sum(u) + c2,  c1 = (1-alpha)*dinv^2, c2 = alpha*dinv*h0.
The MLP encoder (x @ W1.T -> relu -> @ W2.T) runs on the TensorEngine in bf16.
Final epilogue rescales u by sqrt(deg) and applies log_softmax in f32.
"""

import numpy as np

from concourse import bacc, mybir, tile
from concourse.bass import IndirectOffsetOnAxis
from concourse.bass_utils import run_bass_kernel_spmd

AF = mybir.ActivationFunctionType
ALU = mybir.AluOpType
AX = mybir.AxisListType
F32 = mybir.dt.float32
BF16 = mybir.dt.bfloat16
I32 = mybir.dt.int32
BF16_NP = mybir.dt.np(BF16)

P = 128
N_CORES = 8

FULL_CFG = dict(n_nodes=50000, n_feat=512, n_hid=256, n_cls=64, k_hops=10,
                alpha=0.1)


def _host_prep(x, edge_index, W1, W2, cfg):
    """Preprocess graph structure + inputs into per-core device arrays."""
    N = cfg["n_nodes"]
    F = cfg["n_feat"]
    H = cfg["n_hid"]
    C = cfg["n_cls"]
    M = N_CORES
    KC = F // P
    HC = H // P

    src = np.asarray(edge_index[0], dtype=np.int64)
    dst = np.asarray(edge_index[1], dtype=np.int64)
    indeg = np.bincount(dst, minlength=N)
    deg = (indeg + 1).astype(np.float64)        # +1 self loop
    dinv = (1.0 / np.sqrt(deg)).astype(np.float32)
    sqdeg = np.sqrt(deg).astype(np.float32)

    # rank nodes by descending degree; deal round-robin to cores
    order = np.argsort(-deg, kind="stable")     # rank -> old node id
    npc = -(-(-(-N // M) // -1) // P) * P       # ceil(ceil(N/M)/P)*P
    npc = ((N + M - 1) // M + P - 1) // P * P
    nblk = npc // P

    ranks = np.empty(N, np.int64)
    ranks[order] = np.arange(N)
    m_of = (ranks % M).astype(np.int64)
    i_of = ranks // M
    b_of = i_of // P
    p_of = i_of % P
    trow_of = m_of * npc + p_of * nblk + b_of   # table row of each old node
    ZROW = M * npc

    # per-block slot width: max degree over the block across all cores
    deg_mbp = np.zeros((M, nblk, P), np.int64)
    deg_mbp[m_of, b_of, p_of] = deg.astype(np.int64)
    Tb = deg_mbp.max(axis=(0, 2))
    Tb = np.maximum(Tb, 1).astype(np.int64)
    offs = np.zeros(nblk + 1, np.int64)
    np.cumsum(Tb, out=offs[1:])
    sumT = int(offs[-1])

    # CSR of edges by destination (stable keeps duplicates)
    eo = np.argsort(dst, kind="stable")
    s_sorted = src[eo]
    d_sorted = dst[eo]
    indptr = np.zeros(N + 1, np.int64)
    np.cumsum(indeg, out=indptr[1:])

    slots = np.full((M, P, sumT), ZROW, np.int32)
    # self loop at slot 0 of each node
    slots[m_of, p_of, offs[b_of]] = trow_of.astype(np.int32)
    # in-edges at slots 1..deg-1
    pos_in_grp = np.arange(len(d_sorted), dtype=np.int64) - indptr[d_sorted]
    slots[m_of[d_sorted], p_of[d_sorted],
          offs[b_of[d_sorted]] + 1 + pos_in_grp] = trow_of[s_sorted].astype(np.int32)

    # old node at (m, b, p); -1 for padding
    old_at = np.full((M, nblk, P), -1, np.int64)
    old_at[m_of, b_of, p_of] = np.arange(N)

    xf = np.asarray(x, dtype=np.float32)
    in_maps = []
    w1sb = np.ascontiguousarray(
        np.asarray(W1, np.float32).reshape(H, KC, P).transpose(2, 1, 0)
    ).reshape(P, KC * H).astype(BF16_NP)
    w2sb = np.ascontiguousarray(
        np.asarray(W2, np.float32).reshape(C, HC, P).transpose(2, 1, 0)
    ).reshape(P, HC * C).astype(BF16_NP)

    for m in range(M):
        olds = old_at[m].reshape(-1)            # [npc] in (b, p_n) order
        xs = np.zeros((npc, F), np.float32)
        valid = olds >= 0
        xs[valid] = xf[olds[valid]]
        # xsb[p_k, kc*npc + b*P + p_n] = xs[b*P+p_n, kc*P+p_k]
        xsb = np.ascontiguousarray(
            xs.reshape(nblk, P, KC, P).transpose(3, 2, 0, 1)
        ).reshape(P, KC * npc).astype(BF16_NP)

        c1 = np.zeros((P, nblk), np.float32)
        dv = np.zeros((P, nblk), np.float32)
        sq = np.zeros((P, nblk), np.float32)
        mask = m_of == m
        c1[p_of[mask], b_of[mask]] = (1.0 - cfg["alpha"]) * dinv[mask] ** 2
        dv[p_of[mask], b_of[mask]] = dinv[mask]
        sq[p_of[mask], b_of[mask]] = sqdeg[mask]

        in_maps.append({
            "xsb": xsb,
            "w1sb": w1sb,
            "w2sb": w2sb,
            "slots": np.ascontiguousarray(slots[m]),
            "c1": c1,
            "dinv": dv,
            "sqdeg": sq,
        })

    meta = dict(npc=npc, nblk=nblk, Tb=Tb, offs=offs, sumT=sumT,
                m_of=m_of, b_of=b_of, p_of=p_of)
    return in_maps, meta


def _build_nc(cfg, meta):
    N = cfg["n_nodes"]
    F = cfg["n_feat"]
    H = cfg["n_hid"]
    C = cfg["n_cls"]
    K = cfg["k_hops"]
    KC = F // P
    HC = H // P
    npc = meta["npc"]
    nblk = meta["nblk"]
    Tb = meta["Tb"]
    offs = meta["offs"]
    sumT = meta["sumT"]
    Tmax = int(Tb.max())
    NP_ALL = N_CORES * npc
    nrows = NP_ALL + P                          # + zero block
    groups = [list(range(N_CORES))]

    nc = bacc.Bacc("TRN2", target_bir_lowering=False, debug=False,
                   num_devices=N_CORES)

    xsb_d = nc.dram_tensor("xsb", [P, KC * npc], BF16, kind="ExternalInput")
    w1_d = nc.dram_tensor("w1sb", [P, KC * H], BF16, kind="ExternalInput")
    w2_d = nc.dram_tensor("w2sb", [P, HC * C], BF16, kind="ExternalInput")
    slots_d = nc.dram_tensor("slots", [P, sumT], I32, kind="ExternalInput")
    c1_d = nc.dram_tensor("c1", [P, nblk], F32, kind="ExternalInput")
    dinv_d = nc.dram_tensor("dinv", [P, nblk], F32, kind="ExternalInput")
    sqdeg_d = nc.dram_tensor("sqdeg", [P, nblk], F32, kind="ExternalInput")
    out_d = nc.dram_tensor("out", [P, nblk * C], F32, kind="ExternalOutput")

    tables = [nc.dram_tensor(f"table{i}", [nrows, C], BF16) for i in (0, 1)]
    stage_d = nc.dram_tensor("stage", [P, nblk * C], BF16)

    with tile.TileContext(nc) as tc:
        with tc.tile_pool(name="persist", bufs=1) as pp, \
             tc.tile_pool(name="gpool", bufs=2) as gp, \
             tc.tile_pool(name="work", bufs=2) as wp, \
             tc.tile_pool(name="small", bufs=3) as sp, \
             tc.tile_pool(name="psum", bufs=2, space="PSUM") as psp:

            xsb = pp.tile([P, KC * npc], BF16)
            nc.sync.dma_start(out=xsb[:], in_=xsb_d[:])
            w1sb = pp.tile([P, KC * H], BF16)
            nc.sync.dma_start(out=w1sb[:], in_=w1_d[:])
            w2sb = pp.tile([P, HC * C], BF16)
            nc.sync.dma_start(out=w2sb[:], in_=w2_d[:])
            slots = pp.tile([P, sumT], I32)
            nc.sync.dma_start(out=slots[:], in_=slots_d[:])
            c1 = pp.tile([P, nblk], F32)
            nc.sync.dma_start(out=c1[:], in_=c1_d[:])
            dinv = pp.tile([P, nblk], F32)
            nc.sync.dma_start(out=dinv[:], in_=dinv_d[:])
            sqdeg = pp.tile([P, nblk], F32)
            nc.sync.dma_start(out=sqdeg[:], in_=sqdeg_d[:])

            ustage = pp.tile([P, nblk * C], BF16)
            c2 = pp.tile([P, nblk * C], F32)
            ufin = pp.tile([P, nblk * C], F32)
            outst = pp.tile([P, nblk * C], F32)

            zeros = pp.tile([P, C], BF16)
            nc.vector.memset(zeros[:], 0)
            for t in tables:
                nc.sync.dma_start(out=t[NP_ALL:NP_ALL + P, :], in_=zeros[:])

            # ---- MLP encoder: h0 = relu(x @ W1.T) @ W2.T, u0 = dinv * h0 ----
            for b in range(nblk):
                hsb = wp.tile([P, HC * P], BF16, tag="hsb")
                for hh in range(HC):
                    ph = psp.tile([P, P], F32, tag="ph")
                    for kc in range(KC):
                        nc.tensor.matmul(
                            out=ph[:],
                            lhsT=w1sb[:, kc * H + hh * P:kc * H + (hh + 1) * P],
                            rhs=xsb[:, kc * npc + b * P:kc * npc + (b + 1) * P],
                            start=(kc == 0), stop=(kc == KC - 1))
                    nc.scalar.activation(out=hsb[:, hh * P:(hh + 1) * P],
                                         in_=ph[:], func=AF.Relu)
                po = psp.tile([P, C], F32, tag="po")
                for hc in range(HC):
                    nc.tensor.matmul(
                        out=po[:],
                        lhsT=hsb[:, hc * P:(hc + 1) * P],
                        rhs=w2sb[:, hc * C:(hc + 1) * C],
                        start=(hc == 0), stop=(hc == HC - 1))
                dcol = dinv[:, b:b + 1]
                nc.scalar.activation(out=ustage[:, b * C:(b + 1) * C],
                                     in_=po[:], func=AF.Copy, scale=dcol)
                nc.vector.tensor_scalar(
                    out=c2[:, b * C:(b + 1) * C], in0=po[:],
                    scalar1=dcol, scalar2=float(cfg["alpha"]),
                    op0=ALU.mult, op1=ALU.mult)

            nc.sync.dma_start(out=stage_d[:], in_=ustage[:])
            nc.gpsimd.collective_compute(
                "AllGather", ALU.bypass, replica_groups=groups,
                ins=[stage_d[:]], outs=[tables[0][0:NP_ALL, :]])

            # ---- K propagation hops ----
            for k in range(1, K + 1):
                tin = tables[(k - 1) % 2]
                last = (k == K)
                for b in range(nblk):
                    T = int(Tb[b])
                    o = int(offs[b])
                    g = gp.tile([P, Tmax * C], BF16, tag="g")
                    nc.gpsimd.indirect_dma_start(
                        out=g[:, :T * C], out_offset=None,
                        in_=tin[:],
                        in_offset=IndirectOffsetOnAxis(
                            ap=slots[:, o:o + T], axis=0))
                    agg = sp.tile([P, C], F32, tag="agg")
                    nc.vector.tensor_reduce(
                        out=agg[:],
                        in_=g[:, :T * C].rearrange("p (t f) -> p f t", f=C),
                        axis=AX.X, op=ALU.add)
                    tmp = sp.tile([P, C], F32, tag="tmp")
                    nc.scalar.activation(out=tmp[:], in_=agg[:], func=AF.Copy,
                                         scale=c1[:, b:b + 1])
                    dstap = (ufin if last else ustage)[:, b * C:(b + 1) * C]
                    nc.vector.tensor_tensor(out=dstap, in0=tmp[:],
                                            in1=c2[:, b * C:(b + 1) * C],
                                            op=ALU.add)
                if not last:
                    nc.sync.dma_start(out=stage_d[:], in_=ustage[:])
                    nc.gpsimd.collective_compute(
                        "AllGather", ALU.bypass, replica_groups=groups,
                        ins=[stage_d[:]], outs=[tables[k % 2][0:NP_ALL, :]])

            # ---- epilogue: out = log_softmax(u * sqrt(deg)) ----
            for b in range(nblk):
                sc = sp.tile([P, C], F32, tag="sc")
                nc.scalar.activation(out=sc[:], in_=ufin[:, b * C:(b + 1) * C],
                                     func=AF.Copy, scale=sqdeg[:, b:b + 1])
                nmax = sp.tile([P, 1], F32, tag="nmax")
                nc.vector.tensor_reduce(out=nmax[:], in_=sc[:], axis=AX.X,
                                        op=ALU.max, negate=True)
                expd = sp.tile([P, C], F32, tag="expd")
                sume = sp.tile([P, 1], F32, tag="sume")
                nc.scalar.activation(out=expd[:], in_=sc[:], func=AF.Exp,
                                     bias=nmax[:, 0:1], scale=1.0,
                                     accum_out=sume[:])
                lse = sp.tile([P, 1], F32, tag="lse")
                nc.scalar.activation(out=lse[:], in_=sume[:], func=AF.Ln)
                q = sp.tile([P, 1], F32, tag="q")
                nc.vector.tensor_tensor(out=q[:], in0=nmax[:], in1=lse[:],
                                        op=ALU.subtract)
                nc.vector.tensor_scalar(
                    out=outst[:, b * C:(b + 1) * C], in0=sc[:],
                    scalar1=q[:, 0:1], scalar2=None, op0=ALU.add)

            nc.sync.dma_start(out=out_d[:], in_=outst[:])

    nc.compile()
    return nc


def _assemble_output(results, meta, cfg):
    N = cfg["n_nodes"]
    C = cfg["n_cls"]
    nblk = meta["nblk"]
    outs = [np.asarray(r["out"], np.float32).reshape(P, nblk, C)
            for r in results]
    res = np.empty((N, C), np.float32)
    m_of, b_of, p_of = meta["m_of"], meta["b_of"], meta["p_of"]
    stacked = np.stack(outs)                    # [M, P, nblk, C]
    res[:] = stacked[m_of, p_of, b_of]
    return res


def run(inputs, cfg, trace=False):
    in_maps, meta = _host_prep(inputs["x"], inputs["edge_index"],
                               inputs["W1"], inputs["W2"], cfg)
    nc = _build_nc(cfg, meta)
    r = run_bass_kernel_spmd(nc, in_maps, core_ids=list(range(N_CORES)),
                             trace=trace)
    out = _assemble_output(r.results, meta, cfg)
    return out, r


def kernel(**inputs) -> np.ndarray:
    out, _ = run(inputs, FULL_CFG, trace=False)
    return out


# revision 5
# speedup vs baseline: 1.1830x; 1.1830x over previous
"""APPNP GNN (MLP encoder + K-hop personalized-pagerank propagation + log_softmax)
distributed across 8 Trainium2 NeuronCores.

Strategy
--------
Nodes are dealt round-robin by descending degree to the 8 cores (load balance +
uniform per-block gather depth). Each core owns a node shard; the propagation
state u = dinv * out (symmetric-norm folded into per-node scale factors) is
only [N, 64] so every hop we AllGather the bf16 state into a replicated DRAM
table and each core does batched indirect-DMA gathers (one per 128-node block,
slot-padded to the block max degree with pointers at a zero row), a strided
DVE reduction over slots, and a tiny scale+add update:
    u' = c1 * segment_sum(u) + c2,  c1 = (1-alpha)*dinv^2, c2 = alpha*dinv*h0.
The MLP encoder (x @ W1.T -> relu -> @ W2.T) runs on the TensorEngine in bf16.
Final epilogue rescales u by sqrt(deg) and applies log_softmax in f32.
"""

import numpy as np

from concourse import bacc, mybir, tile
from concourse.bass import IndirectOffsetOnAxis
from concourse.bass_utils import run_bass_kernel_spmd
from concourse.masks import make_identity

AF = mybir.ActivationFunctionType
ALU = mybir.AluOpType
AX = mybir.AxisListType
F32 = mybir.dt.float32
BF16 = mybir.dt.bfloat16
I32 = mybir.dt.int32
BF16_NP = mybir.dt.np(BF16)

P = 128
N_CORES = 8

FULL_CFG = dict(n_nodes=50000, n_feat=512, n_hid=256, n_cls=64, k_hops=10,
                alpha=0.1)


def _host_prep(x, edge_index, W1, W2, cfg):
    """Preprocess graph structure + inputs into per-core device arrays."""
    N = cfg["n_nodes"]
    F = cfg["n_feat"]
    H = cfg["n_hid"]
    C = cfg["n_cls"]
    M = N_CORES
    KC = F // P
    HC = H // P

    src = np.asarray(edge_index[0], dtype=np.int64)
    dst = np.asarray(edge_index[1], dtype=np.int64)
    indeg = np.bincount(dst, minlength=N)
    deg = (indeg + 1).astype(np.float64)        # +1 self loop
    dinv = (1.0 / np.sqrt(deg)).astype(np.float32)
    sqdeg = np.sqrt(deg).astype(np.float32)

    # rank nodes by descending degree; deal round-robin to cores
    order = np.argsort(-deg, kind="stable")     # rank -> old node id
    npc = -(-(-(-N // M) // -1) // P) * P       # ceil(ceil(N/M)/P)*P
    npc = ((N + M - 1) // M + P - 1) // P * P
    nblk = npc // P

    ranks = np.empty(N, np.int64)
    ranks[order] = np.arange(N)
    m_of = (ranks % M).astype(np.int64)
    i_of = ranks // M
    b_of = i_of // P
    p_of = i_of % P
    trow_of = m_of * npc + p_of * nblk + b_of   # table row of each old node
    ZROW = M * npc

    # per-block slot width: max degree over the block across all cores
    deg_mbp = np.zeros((M, nblk, P), np.int64)
    deg_mbp[m_of, b_of, p_of] = deg.astype(np.int64)
    Tb = deg_mbp.max(axis=(0, 2))
    Tb = np.maximum(Tb, 1).astype(np.int64)
    offs = np.zeros(nblk + 1, np.int64)
    np.cumsum(Tb, out=offs[1:])
    sumT = int(offs[-1])

    # CSR of edges by destination (stable keeps duplicates)
    eo = np.argsort(dst, kind="stable")
    s_sorted = src[eo]
    d_sorted = dst[eo]
    indptr = np.zeros(N + 1, np.int64)
    np.cumsum(indeg, out=indptr[1:])

    slots = np.full((M, P, sumT), ZROW, np.int32)
    # self loop at slot 0 of each node
    slots[m_of, p_of, offs[b_of]] = trow_of.astype(np.int32)
    # in-edges at slots 1..deg-1
    pos_in_grp = np.arange(len(d_sorted), dtype=np.int64) - indptr[d_sorted]
    slots[m_of[d_sorted], p_of[d_sorted],
          offs[b_of[d_sorted]] + 1 + pos_in_grp] = trow_of[s_sorted].astype(np.int32)

    # old node at (m, b, p); -1 for padding
    old_at = np.full((M, nblk, P), -1, np.int64)
    old_at[m_of, b_of, p_of] = np.arange(N)

    xf = np.asarray(x, dtype=np.float32)
    in_maps = []
    w1sb = np.ascontiguousarray(
        np.asarray(W1, np.float32).reshape(H, KC, P).transpose(2, 1, 0)
    ).reshape(P, KC * H).astype(BF16_NP)
    w2sb = np.ascontiguousarray(
        np.asarray(W2, np.float32).reshape(C, HC, P).transpose(2, 1, 0)
    ).reshape(P, HC * C).astype(BF16_NP)

    for m in range(M):
        olds = old_at[m].reshape(-1)            # [npc] in (b, p_n) order
        xs = np.zeros((npc, F), np.float32)
        valid = olds >= 0
        xs[valid] = xf[olds[valid]]
        # xsb[p_k, kc*npc + b*P + p_n] = xs[b*P+p_n, kc*P+p_k]
        xsb = np.ascontiguousarray(
            xs.reshape(nblk, P, KC, P).transpose(3, 2, 0, 1)
        ).reshape(P, KC * npc).astype(BF16_NP)

        c1 = np.zeros((P, nblk), np.float32)
        dv = np.zeros((P, nblk), np.float32)
        sq = np.zeros((P, nblk), np.float32)
        mask = m_of == m
        c1[p_of[mask], b_of[mask]] = (1.0 - cfg["alpha"]) * dinv[mask] ** 2
        dv[p_of[mask], b_of[mask]] = dinv[mask]
        sq[p_of[mask], b_of[mask]] = sqdeg[mask]

        in_maps.append({
            "xsb": xsb,
            "w1sb": w1sb,
            "w2sb": w2sb,
            "slots": np.ascontiguousarray(slots[m]),
            "c1": c1,
            "dinv": dv,
            "sqdeg": sq,
        })

    meta = dict(npc=npc, nblk=nblk, Tb=Tb, offs=offs, sumT=sumT,
                m_of=m_of, b_of=b_of, p_of=p_of)
    return in_maps, meta


def _build_nc(cfg, meta):
    N = cfg["n_nodes"]
    F = cfg["n_feat"]
    H = cfg["n_hid"]
    C = cfg["n_cls"]
    K = cfg["k_hops"]
    KC = F // P
    HC = H // P
    npc = meta["npc"]
    nblk = meta["nblk"]
    Tb = meta["Tb"]
    offs = meta["offs"]
    sumT = meta["sumT"]
    Tmax = int(Tb.max())
    NP_ALL = N_CORES * npc
    nrows = NP_ALL + P                          # + zero block
    groups = [list(range(N_CORES))]

    nc = bacc.Bacc("TRN2", target_bir_lowering=False, debug=False,
                   num_devices=N_CORES)

    xsb_d = nc.dram_tensor("xsb", [P, KC * npc], BF16, kind="ExternalInput")
    w1_d = nc.dram_tensor("w1sb", [P, KC * H], BF16, kind="ExternalInput")
    w2_d = nc.dram_tensor("w2sb", [P, HC * C], BF16, kind="ExternalInput")
    slots_d = nc.dram_tensor("slots", [P, sumT], I32, kind="ExternalInput")
    c1_d = nc.dram_tensor("c1", [P, nblk], F32, kind="ExternalInput")
    dinv_d = nc.dram_tensor("dinv", [P, nblk], F32, kind="ExternalInput")
    sqdeg_d = nc.dram_tensor("sqdeg", [P, nblk], F32, kind="ExternalInput")
    out_d = nc.dram_tensor("out", [P, nblk * C], F32, kind="ExternalOutput")

    tables = [nc.dram_tensor(f"table{i}", [nrows, C], BF16, addr_space="Shared")
              for i in (0, 1)]
    stage_d = nc.dram_tensor("stage", [P, nblk * C], BF16)

    with tile.TileContext(nc) as tc:
        with tc.tile_pool(name="persist", bufs=1) as pp, \
             tc.tile_pool(name="gpool", bufs=2) as gp, \
             tc.tile_pool(name="work", bufs=2) as wp, \
             tc.tile_pool(name="small", bufs=3) as sp, \
             tc.tile_pool(name="psum", bufs=2, space="PSUM") as psp:

            xsb = pp.tile([P, KC * npc], BF16)
            nc.sync.dma_start(out=xsb[:], in_=xsb_d[:])
            w1sb = pp.tile([P, KC * H], BF16)
            nc.sync.dma_start(out=w1sb[:], in_=w1_d[:])
            w2sb = pp.tile([P, HC * C], BF16)
            nc.sync.dma_start(out=w2sb[:], in_=w2_d[:])
            slots = pp.tile([P, sumT], I32)
            nc.sync.dma_start(out=slots[:], in_=slots_d[:])
            c1 = pp.tile([P, nblk], F32)
            nc.sync.dma_start(out=c1[:], in_=c1_d[:])
            dinv = pp.tile([P, nblk], F32)
            nc.sync.dma_start(out=dinv[:], in_=dinv_d[:])
            sqdeg = pp.tile([P, nblk], F32)
            nc.sync.dma_start(out=sqdeg[:], in_=sqdeg_d[:])

            ustage = pp.tile([P, nblk * C], BF16)
            c2 = pp.tile([P, nblk * C], F32)
            ufin = pp.tile([P, nblk * C], F32)
            outst = pp.tile([P, nblk * C], F32)

            zeros = pp.tile([P, C], BF16)
            nc.vector.memset(zeros[:], 0)
            for t in tables:
                nc.sync.dma_start(out=t[NP_ALL:NP_ALL + P, :], in_=zeros[:])
            ident = pp.tile([P, P], BF16)
            make_identity(nc, ident[:])

            # ---- MLP encoder: h0 = relu(x @ W1.T) @ W2.T, u0 = dinv * h0 ----
            for b in range(nblk):
                hsb = wp.tile([P, HC * P], BF16, tag="hsb")
                for hh in range(HC):
                    ph = psp.tile([P, P], F32, tag="ph")
                    for kc in range(KC):
                        nc.tensor.matmul(
                            out=ph[:],
                            lhsT=w1sb[:, kc * H + hh * P:kc * H + (hh + 1) * P],
                            rhs=xsb[:, kc * npc + b * P:kc * npc + (b + 1) * P],
                            start=(kc == 0), stop=(kc == KC - 1))
                    nc.scalar.activation(out=hsb[:, hh * P:(hh + 1) * P],
                                         in_=ph[:], func=AF.Relu)
                po = psp.tile([P, C], F32, tag="po")
                for hc in range(HC):
                    nc.tensor.matmul(
                        out=po[:],
                        lhsT=hsb[:, hc * P:(hc + 1) * P],
                        rhs=w2sb[:, hc * C:(hc + 1) * C],
                        start=(hc == 0), stop=(hc == HC - 1))
                dcol = dinv[:, b:b + 1]
                nc.scalar.activation(out=ustage[:, b * C:(b + 1) * C],
                                     in_=po[:], func=AF.Copy, scale=dcol)
                nc.vector.tensor_scalar(
                    out=c2[:, b * C:(b + 1) * C], in0=po[:],
                    scalar1=dcol, scalar2=float(cfg["alpha"]),
                    op0=ALU.mult, op1=ALU.mult)

            nc.sync.dma_start(out=stage_d[:], in_=ustage[:])
            nc.gpsimd.collective_compute(
                "AllGather", ALU.bypass, replica_groups=groups,
                ins=[stage_d[:]], outs=[tables[0][0:NP_ALL, :]])

            # ---- K propagation hops ----
            for k in range(1, K + 1):
                tin = tables[(k - 1) % 2]
                last = (k == K)
                for b in range(nblk):
                    T = int(Tb[b])
                    o = int(offs[b])
                    g = gp.tile([P, Tmax * C], BF16, tag="g")
                    nc.gpsimd.indirect_dma_start(
                        out=g[:, :T * C], out_offset=None,
                        in_=tin[:],
                        in_offset=IndirectOffsetOnAxis(
                            ap=slots[:, o:o + T], axis=0))
                    pg = psp.tile([P, C], F32, tag="pg")
                    for t in range(T):
                        nc.tensor.matmul(out=pg[:], lhsT=ident[:],
                                         rhs=g[:, t * C:(t + 1) * C],
                                         start=(t == 0), stop=(t == T - 1))
                    tmp = sp.tile([P, C], F32, tag="tmp")
                    nc.scalar.activation(out=tmp[:], in_=pg[:], func=AF.Copy,
                                         scale=c1[:, b:b + 1])
                    dstap = (ufin if last else ustage)[:, b * C:(b + 1) * C]
                    nc.vector.tensor_tensor(out=dstap, in0=tmp[:],
                                            in1=c2[:, b * C:(b + 1) * C],
                                            op=ALU.add)
                if not last:
                    nc.sync.dma_start(out=stage_d[:], in_=ustage[:])
                    nc.gpsimd.collective_compute(
                        "AllGather", ALU.bypass, replica_groups=groups,
                        ins=[stage_d[:]], outs=[tables[k % 2][0:NP_ALL, :]])

            # ---- epilogue: out = log_softmax(u * sqrt(deg)) ----
            for b in range(nblk):
                sc = sp.tile([P, C], F32, tag="sc")
                nc.scalar.activation(out=sc[:], in_=ufin[:, b * C:(b + 1) * C],
                                     func=AF.Copy, scale=sqdeg[:, b:b + 1])
                nmax = sp.tile([P, 1], F32, tag="nmax")
                nc.vector.tensor_reduce(out=nmax[:], in_=sc[:], axis=AX.X,
                                        op=ALU.max, negate=True)
                expd = sp.tile([P, C], F32, tag="expd")
                sume = sp.tile([P, 1], F32, tag="sume")
                nc.scalar.activation(out=expd[:], in_=sc[:], func=AF.Exp,
                                     bias=nmax[:, 0:1], scale=1.0,
                                     accum_out=sume[:])
                lse = sp.tile([P, 1], F32, tag="lse")
                nc.scalar.activation(out=lse[:], in_=sume[:], func=AF.Ln)
                q = sp.tile([P, 1], F32, tag="q")
                nc.vector.tensor_tensor(out=q[:], in0=nmax[:], in1=lse[:],
                                        op=ALU.subtract)
                nc.vector.tensor_scalar(
                    out=outst[:, b * C:(b + 1) * C], in0=sc[:],
                    scalar1=q[:, 0:1], scalar2=None, op0=ALU.add)

            nc.sync.dma_start(out=out_d[:], in_=outst[:])

    nc.compile()
    return nc


def _assemble_output(results, meta, cfg):
    N = cfg["n_nodes"]
    C = cfg["n_cls"]
    nblk = meta["nblk"]
    outs = [np.asarray(r["out"], np.float32).reshape(P, nblk, C)
            for r in results]
    res = np.empty((N, C), np.float32)
    m_of, b_of, p_of = meta["m_of"], meta["b_of"], meta["p_of"]
    stacked = np.stack(outs)                    # [M, P, nblk, C]
    res[:] = stacked[m_of, p_of, b_of]
    return res


def run(inputs, cfg, trace=False):
    in_maps, meta = _host_prep(inputs["x"], inputs["edge_index"],
                               inputs["W1"], inputs["W2"], cfg)
    nc = _build_nc(cfg, meta)
    r = run_bass_kernel_spmd(nc, in_maps, core_ids=list(range(N_CORES)),
                             trace=trace)
    out = _assemble_output(r.results, meta, cfg)
    return out, r


def kernel(**inputs) -> np.ndarray:
    out, _ = run(inputs, FULL_CFG, trace=False)
    return out
